# revision 1
# baseline (speedup 1.0000x reference)
"""ETC transient-global self-attention on 8 TRN2 NeuronCores.

Sharding: sequence-parallel. Core c handles example b = c//4, token rows
[1024*(c%4), 1024*(c%4+1)). Each core computes its q/k/v projections (k/v
with a 1-block halo), the per-example global (side) aggregates from the
full example, local+global attention, and the output projection for its
token rows. No cross-core communication; the host stacks the 8 row-slices.

Shapes (hardcoded from the problem spec):
  x  [2, 4096, 1024], Wq/Wk/Wv [1024, 16, 64], Wo [16, 64, 1024]
  block_len 128, 32 blocks, TOKENS_PER_BLOCK 16 -> G = 256 side tokens.

On-device layout notes:
  - everything runs in bf16 on the PE (f32 accumulate in PSUM).
  - attention logits are computed TRANSPOSED ([keys, q]) so that exp() is
    orientation-agnostic and PV contracts keys on the partition dim with no
    probs transpose. Softmax denominators come from a ones-column appended
    to v (PSUM row 64); the reference's extra-logit softmax needs no
    max-subtraction since logits are O(1) and masked entries multiply to 0.
"""

import numpy as np
import ml_dtypes

B, L, D, H, DH = 2, 4096, 1024, 16, 64
BL = 128                 # block length
NBLK = 32                # total blocks
G = 256                  # side (global) tokens
TPB = 16                 # tokens per side block
N_CORES = 8
NB = 8                   # blocks per core
TOK = NB * BL            # 1024 q tokens per core
KV = TOK + 2 * BL        # 1280 kv tokens (1-block halo each side)
BF16 = ml_dtypes.bfloat16

_PROG = None


def _build_program():
    import concourse.bass as bass
    import concourse.mybir as mybir
    import concourse.tile as tile
    from concourse import bacc
    from concourse.masks import make_identity

    dt = mybir.dt
    f32, bf16 = dt.float32, dt.bfloat16

    nc = bacc.Bacc("TRN2", target_bir_lowering=False, debug=False,
                   num_devices=N_CORES)

    xkv_d = nc.declare_dram_parameter("xkv", [KV, D], bf16, isOutput=False)
    g_d = nc.declare_dram_parameter("g", [G, D], bf16, isOutput=False)
    wq_d = nc.declare_dram_parameter("wq", [D, D], bf16, isOutput=False)
    wk_d = nc.declare_dram_parameter("wk", [D, D], bf16, isOutput=False)
    wv_d = nc.declare_dram_parameter("wv", [D, D], bf16, isOutput=False)
    wo_d = nc.declare_dram_parameter("wo", [D, D], bf16, isOutput=False)
    msk_d = nc.declare_dram_parameter("maskT", [BL, NB, 3, BL], bf16,
                                      isOutput=False)
    out_d = nc.declare_dram_parameter("out", [TOK, D], f32, isOutput=True)

    Exp = mybir.ActivationFunctionType.Exp
    Cpy = mybir.ActivationFunctionType.Copy

    with tile.TileContext(nc) as tc:
        with (
            tc.tile_pool(name="per", bufs=1) as per,
            tc.tile_pool(name="strm", bufs=3) as strm,
            tc.tile_pool(name="att", bufs=2) as att,
            tc.tile_pool(name="osb", bufs=2) as osb,
        ):
            # ---- persistent SBUF tiles ----
            wk_sb = per.tile([128, 8, D], bf16, tag="wk")
            wv_sb = per.tile([128, 8, D], bf16, tag="wv")
            wo_sb = per.tile([128, 8, D], bf16, tag="wo")
            wq_sb = per.tile([128, 8, D], bf16, tag="wq_yT")   # dies -> yTf
            msk_sb = per.tile([128, NB, 3, BL], bf16, tag="msk")
            ident = per.tile([128, 128], bf16, tag="ident")
            ones64 = per.tile([128, 64], bf16, tag="ones64")
            gnat = per.tile([128, 2, D], bf16, tag="gnat")
            gT = per.tile([128, 8, G], bf16, tag="gT")
            skT = per.tile([128, 8, G], bf16, tag="skT")
            svaug = per.tile([128, 2, H, DH + 1], bf16, tag="svaug")
            qT = per.tile([128, 8, TOK], bf16, tag="qT")
            kT = per.tile([128, 8, KV], bf16, tag="kT")
            vaug = per.tile([128, 10, H, DH + 1], bf16, tag="vaug")
            xT = per.tile([128, 8, KV], bf16, tag="xT_st")     # dies -> stage_o

            # DMA issue order matters for ramp-up: wk/wq land first so the
            # first kT accumulation group can chase the xT transpose chunks
            # as they arrive instead of waiting for the whole queue.
            nc.sync.dma_start(out=wq_sb,
                              in_=wq_d.ap().rearrange("(c p) d -> p c d", p=128))
            # xT built directly by 2-byte transpose DMAs (one per D-chunk)
            for dc in range(8):
                nc.sync.dma_start(out=xT[:, dc, :],
                                  in_=xkv_d[:, dc * 128:(dc + 1) * 128],
                                  transpose=True)
            nc.sync.dma_start(out=wv_sb,
                              in_=wv_d.ap().rearrange("(c p) d -> p c d", p=128))
            nc.sync.dma_start(out=gnat,
                              in_=g_d.ap().rearrange("(t p) d -> p t d", p=128))
            nc.sync.dma_start(out=msk_sb, in_=msk_d.ap())
            nc.sync.dma_start(out=wk_sb,
                              in_=wk_d.ap().rearrange("(c p) d -> p c d", p=128))
            nc.sync.dma_start(out=wo_sb,
                              in_=wo_d.ap().rearrange("(c p) d -> p c d", p=128))
            make_identity(nc, ident)
            nc.vector.memset(ones64, 1.0)

            with tc.tile_pool(name="pst", bufs=2, space="PSUM") as pst:
                # ---- build gT (xT comes straight from transpose DMA) ----
                for gt_i in range(2):
                    for dc in range(8):
                        pt = pst.tile([128, 128], bf16, tag="tp")
                        nc.tensor.transpose(
                            pt, gnat[:, gt_i, dc * 128:(dc + 1) * 128], ident)
                        nc.scalar.copy(gT[:, dc, gt_i * 128:(gt_i + 1) * 128], pt)

            # Projections (except kT) run first; the kT projection for each
            # head pair is interleaved with that pair's attention so the
            # PE-bound projection work overlaps the ACT-bound exp window.
            with tc.tile_pool(name="pspa", bufs=2, space="PSUM") as psp:
                # qT (q tokens = xT kv-rows 128..1152), Wq pre-scaled by 1/8.
                # Emitted first so wq dies before yTf reuses its SBUF slot.
                for oc in range(8):
                    for tch in range(2):
                        ts_ = 128 + tch * 512
                        pp = psp.tile([128, 512], f32, tag="pj")
                        for dc in range(8):
                            nc.tensor.matmul(
                                pp,
                                wq_sb[:, dc, oc * 128:(oc + 1) * 128],
                                xT[:, dc, ts_:ts_ + 512],
                                start=(dc == 0), stop=(dc == 7))
                        nc.vector.tensor_copy(
                            qT[:, oc, tch * 512:(tch + 1) * 512], pp)
                # v natural (augmented with ones column per head)
                for t in range(10):
                    for j in range(2):
                        pp = psp.tile([128, 512], f32, tag="pj")
                        for dc in range(8):
                            nc.tensor.matmul(
                                pp,
                                xT[:, dc, t * 128:(t + 1) * 128],
                                wv_sb[:, dc, 512 * j:512 * (j + 1)],
                                start=(dc == 0), stop=(dc == 7))
                        nc.scalar.copy(
                            vaug[:, t, 8 * j:8 * (j + 1), 0:DH],
                            pp.rearrange("p (h d) -> p h d", h=8))
                    nc.vector.memset(vaug[:, t, :, DH:DH + 1], 1.0)
                # side kT
                for oc in range(8):
                    pp = psp.tile([128, 512], f32, tag="pj")
                    for dc in range(8):
                        nc.tensor.matmul(
                            pp[:, :G],
                            wk_sb[:, dc, oc * 128:(oc + 1) * 128],
                            gT[:, dc, :],
                            start=(dc == 0), stop=(dc == 7))
                    nc.vector.tensor_copy(skT[:, oc, :], pp[:, :G])
                # side v (augmented)
                for gt_i in range(2):
                    for j in range(2):
                        pp = psp.tile([128, 512], f32, tag="pj")
                        for dc in range(8):
                            nc.tensor.matmul(
                                pp,
                                gT[:, dc, gt_i * 128:(gt_i + 1) * 128],
                                wv_sb[:, dc, 512 * j:512 * (j + 1)],
                                start=(dc == 0), stop=(dc == 7))
                        nc.scalar.copy(
                            svaug[:, gt_i, 8 * j:8 * (j + 1), 0:DH],
                            pp.rearrange("p (h d) -> p h d", h=8))
                    nc.vector.memset(svaug[:, gt_i, :, DH:DH + 1], 1.0)

            # ---- kT projection + attention, interleaved per head pair
            with tc.tile_pool(name="psp", bufs=1, space="PSUM") as psp, \
                 tc.tile_pool(name="plg", bufs=2, space="PSUM") as plg, \
                 tc.tile_pool(name="psg2", bufs=1, space="PSUM") as psg2, \
                 tc.tile_pool(name="pyt", bufs=2, space="PSUM") as pyt, \
                 tc.tile_pool(name="pbc", bufs=1, space="PSUM") as pbc:
                yTf = per.tile([128, 8, TOK], bf16, tag="wq_yT")

                def attn_head(h, stg):
                    oc, r0 = h // 2, 64 * (h % 2)
                    for nh in range(2):          # half = 4 blocks = 512 q
                        q4 = qT[r0:r0 + 64, oc, nh * 512:(nh + 1) * 512]
                        # side QK batched over the 4 blocks (N=512), one exp
                        sg = psg2.tile([128, 2, 512], f32, tag="sg",
                                       name=f"sg{h}_{nh}")
                        us = att.tile([128, 2, 512], bf16, tag="us", bufs=3,
                                      name=f"us{h}_{nh}")
                        for g in range(2):
                            nc.tensor.matmul(
                                sg[:, g, :],
                                skT[r0:r0 + 64, oc, g * 128:(g + 1) * 128],
                                q4, start=True, stop=True)
                        nc.scalar.activation(us, sg, Exp)
                        # local QK + exp + mask per block
                        uls = []
                        for i in range(4):
                            n = nh * 4 + i
                            qs = qT[r0:r0 + 64, oc, n * 128:(n + 1) * 128]
                            lg = plg.tile([128, 3, 128], f32, tag="lg",
                                          name=f"lg{h}_{n}")
                            for c in range(3):
                                nc.tensor.matmul(
                                    lg[:, c, :],
                                    kT[r0:r0 + 64, oc,
                                       (n + c) * 128:(n + c + 1) * 128],
                                    qs, start=True, stop=True)
                            ul = att.tile([128, 3, 128], bf16, tag="ul", bufs=8,
                                          name=f"ul{h}_{n}")
                            nc.scalar.activation(ul, lg, Exp)
                            nc.vector.tensor_mul(ul, ul, msk_sb[:, n, :, :])
                            uls.append(ul)
                        # PV for 4 blocks into one [65, 512] psum
                        yt = pyt.tile([65, 512], f32, tag="yt",
                                      name=f"yt{h}_{nh}")
                        for i in range(4):
                            n = nh * 4 + i
                            for c in range(3):
                                # start=True clears this whole PSUM bank, so
                                # only the very first matmul of the group may
                                # set it; per-element has_written handles the
                                # first write of each column slice.
                                nc.tensor.matmul(
                                    yt[:, i * 128:(i + 1) * 128],
                                    vaug[:, n + c, h, :], uls[i][:, c, :],
                                    start=(i == 0 and c == 0), stop=False)
                        for g in range(2):
                            nc.tensor.matmul(
                                yt, svaug[:, g, h, :], us[:, g, :],
                                start=False, stop=(g == 1))
                        # normalize: bcast (denom+1), lane-parallel recip, mul
                        rc = att.tile([128, 512], bf16, tag="rc", bufs=2,
                                      name=f"rc{h}_{nh}")
                        nc.vector.tensor_scalar_add(rc[64:65, :], yt[64:65, :],
                                                    1.0)
                        bc = pbc.tile([64, 512], f32, tag="bc",
                                      name=f"bc{h}_{nh}")
                        nc.tensor.matmul(bc, ones64[64:65, :], rc[64:65, :],
                                         start=True, stop=True)
                        rcb = att.tile([64, 512], bf16, tag="rcb", bufs=2,
                                       name=f"rcb{h}_{nh}")
                        with nc.allow_low_precision(reason="bf16 softmax recip"):
                            nc.vector.reciprocal(rcb, bc)
                        dst = (yTf[0:64, oc, nh * 512:(nh + 1) * 512]
                               if h % 2 == 0 else
                               stg[:, nh * 512:(nh + 1) * 512])
                        nc.vector.tensor_mul(dst, yt[0:64, :], rcb)

                for oc in range(8):
                    # kT projection for this head pair
                    for ts_, te in ((0, 512), (512, 1024), (1024, 1280)):
                        pp = psp.tile([128, 512], f32, tag="pj",
                                      name=f"ppk{oc}_{ts_}")
                        for dc in range(8):
                            nc.tensor.matmul(
                                pp[:, :te - ts_],
                                wk_sb[:, dc, oc * 128:(oc + 1) * 128],
                                xT[:, dc, ts_:te],
                                start=(dc == 0), stop=(dc == 7))
                        nc.vector.tensor_copy(kT[:, oc, ts_:te],
                                              pp[:, :te - ts_])
                    attn_head(2 * oc, None)
                    stg = att.tile([64, TOK], bf16, tag="stg", bufs=2,
                                   name=f"stg{oc}")
                    attn_head(2 * oc + 1, stg)
                    # shift this pair's odd head up to partitions 64..127
                    nc.sync.dma_start(out=yTf[64:128, oc, :], in_=stg)

            # ---- phase F: output projection ----
            with tc.tile_pool(name="pso", bufs=2, space="PSUM") as pso:
                for tt in range(8):
                    ot = osb.tile([128, D], f32, tag="ot")
                    for j in range(2):
                        pp = pso.tile([128, 512], f32, tag="po")
                        for oc in range(8):
                            nc.tensor.matmul(
                                pp,
                                yTf[:, oc, tt * 128:(tt + 1) * 128],
                                wo_sb[:, oc, 512 * j:512 * (j + 1)],
                                start=(oc == 0), stop=(oc == 7))
                        nc.vector.tensor_copy(ot[:, 512 * j:512 * (j + 1)], pp)
                    nc.sync.dma_start(out=out_d[tt * 128:(tt + 1) * 128, :],
                                      in_=ot)

    nc.compile()
    return nc


def _host_inputs(x, Wq, Wk, Wv, Wo):
    """Build the 8 per-core input maps (all numpy, bf16 where device expects)."""
    xbf = x.astype(BF16)
    wq = (Wq.reshape(D, D).astype(np.float32) / np.sqrt(DH)).astype(BF16)
    wk = Wk.reshape(D, D).astype(BF16)
    wv = Wv.reshape(D, D).astype(BF16)
    wo = Wo.reshape(D, D).astype(BF16)

    # per-example side aggregates (sum of x over 16-token groups), f32 sum
    g_all = x.reshape(B, G, TPB, D).sum(2).astype(BF16)

    in_maps = []
    for c in range(N_CORES):
        b, s = c // 4, c % 4
        S0 = s * TOK
        blk0 = S0 // BL
        xkv = np.zeros((KV, D), BF16)
        a0 = S0 - BL
        lo, hi = max(a0, 0), min(a0 + KV, L)
        xkv[lo - a0:hi - a0] = xbf[b, lo:hi]
        # maskT[k, n, c, q]: local-window validity, transposed
        k_ = np.arange(BL)[:, None, None, None]
        n_ = np.arange(NB)[None, :, None, None]
        c_ = np.arange(3)[None, None, :, None]
        q_ = np.arange(BL)[None, None, None, :]
        rel = (c_ * BL + k_) - BL - q_
        kpos = (blk0 + n_ - 1) * BL + c_ * BL + k_
        valid = (np.abs(rel) <= BL - 1) & (kpos >= 0) & (kpos < L)
        in_maps.append({
            "xkv": xkv,
            "g": g_all[b],
            "wq": wq, "wk": wk, "wv": wv, "wo": wo,
            "maskT": valid.astype(BF16),
        })
    return in_maps


_RUNNER = None


def _make_runner(nc):
    """Build the PJRT executable once; returns fn(in_maps) -> per-core outs.

    Mirrors concourse.bass2jax.run_bass_via_pjrt, but caches the jitted
    shard_map callable so repeat kernel() calls skip retrace/recompile.
    """
    import jax
    import numpy as _np
    from jax.sharding import Mesh, PartitionSpec
    from jax.experimental.shard_map import shard_map
    import concourse.mybir as mybir
    from concourse import bass2jax

    bass2jax.install_neuronx_cc_hook()
    partition_name = (nc.partition_id_tensor.name
                      if nc.partition_id_tensor else None)
    in_names, out_names, out_avals = [], [], []
    for alloc in nc.m.functions[0].allocations:
        if not isinstance(alloc, mybir.MemoryLocationSet):
            continue
        name = alloc.memorylocations[0].name
        if alloc.kind == "ExternalInput":
            if name != partition_name:
                in_names.append(name)
        elif alloc.kind == "ExternalOutput":
            out_avals.append(jax.core.ShapedArray(
                tuple(alloc.tensor_shape), mybir.dt.np(alloc.dtype)))
            out_names.append(name)
    n_params = len(in_names)
    all_names = in_names + out_names
    if partition_name is not None:
        all_names.append(partition_name)
    donate = tuple(range(n_params, n_params + len(out_names)))

    def _body(*args):
        operands = list(args)
        if partition_name is not None:
            operands.append(bass2jax.partition_id_tensor())
        return tuple(bass2jax._bass_exec_p.bind(
            *operands, out_avals=tuple(out_avals), in_names=tuple(all_names),
            out_names=tuple(out_names), lowering_input_output_aliases=(),
            sim_require_finite=True, sim_require_nnan=True, nc=nc))

    devices = jax.devices()[:N_CORES]
    mesh = Mesh(_np.asarray(devices), ("core",))
    specs = (PartitionSpec("core"),) * (n_params + len(out_names))
    sharded = jax.jit(
        shard_map(_body, mesh=mesh, in_specs=specs,
                  out_specs=(PartitionSpec("core"),) * len(out_names),
                  check_rep=False),
        donate_argnums=donate, keep_unused=True)

    def run(in_maps):
        concat_in = [
            _np.concatenate([_np.asarray(in_maps[c][k]) for c in range(N_CORES)],
                            axis=0)
            for k in in_names
        ]
        concat_zeros = [_np.zeros((N_CORES * a.shape[0], *a.shape[1:]), a.dtype)
                        for a in out_avals]
        outs = sharded(*concat_in, *concat_zeros)
        return [
            {k: _np.asarray(outs[i]).reshape(N_CORES, *out_avals[i].shape)[c]
             for i, k in enumerate(out_names)}
            for c in range(N_CORES)
        ]

    return run


def kernel(x, Wq, Wk, Wv, Wo):
    global _PROG, _RUNNER
    if _RUNNER is None:
        _PROG = _build_program()
        _RUNNER = _make_runner(_PROG)
    in_maps = _host_inputs(np.asarray(x, np.float32), np.asarray(Wq, np.float32),
                           np.asarray(Wk, np.float32), np.asarray(Wv, np.float32),
                           np.asarray(Wo, np.float32))
    results = _RUNNER(in_maps)
    out = np.empty((B, L, D), np.float32)
    for c in range(N_CORES):
        b, s = c // 4, c % 4
        out[b, s * TOK:(s + 1) * TOK] = results[c]["out"]
    return out



# revision 2
# speedup vs baseline: 15690.1275x; 15690.1275x over previous
"""ETC transient-global self-attention on 8 TRN2 NeuronCores.

Sharding: sequence-parallel. Core c handles example b = c//4, token rows
[1024*(c%4), 1024*(c%4+1)). Each core computes q/k/v projections (k/v with a
1-block halo), per-example global (side) aggregates, local+global attention,
and the output projection for its token rows. No cross-core communication;
the host stacks the 8 row-slices.

Shapes (hardcoded): x [2, 4096, 1024], Wq/Wk/Wv [1024, 16, 64],
Wo [16, 64, 1024]; block_len 128, TOKENS_PER_BLOCK 16 -> G = 256.

Device-side numerics:
  - All projections run as fp8e4m3 DoubleRow "split" matmuls: each operand is
    host-decomposed into hi + lo e4m3 parts (after a power-of-2 scale that
    keeps values out of the subnormal range), and the product is computed as
    hi*w_hi (chunk-paired) + (hi*w_lo + lo*w_hi) per chunk. That is ~bf16
    precision at 0.75x the bf16 PE-column cost.
  - x is pre-transposed on the host (no device transposes).
  - Attention runs in bf16 exactly as the reference (transposed logits,
    exp via ACT, masked c0/c2 chunks only, PV with a ones-column producing
    softmax denominators in PSUM row 64). The denominator reciprocal is
    broadcast across partitions with a GpSimd partition_broadcast.
  - Output is DMA'd out in bf16 and upcast on the host.
"""

import numpy as np
import ml_dtypes

B, L, D, H, DH = 2, 4096, 1024, 16, 64
BL = 128                 # block length
G = 256                  # side (global) tokens per example
TPB = 16                 # tokens per side block
N_CORES = 8
NB = 8                   # blocks per core
TOK = NB * BL            # 1024 q tokens per core
KV = TOK + 2 * BL        # 1280 kv tokens (1-block halo each side)
BF16 = ml_dtypes.bfloat16
F8 = ml_dtypes.float8_e4m3

SX = 16.0                # x scale before fp8 split
SW = 512.0               # weight scale before fp8 split
SG = 4.0                 # side-aggregate scale before fp8 split
QS = 1.0 / (SX * SW * 8.0)   # qT copy scale (wq also pre-divided by sqrt(dh))
KS = 1.0 / (SX * SW)         # kT/v copy scale
GS = 1.0 / (SG * SW)         # side k/v copy scale

_PROG = None


def _build_program():
    import concourse.mybir as mybir
    import concourse.tile as tile
    from concourse import bacc

    dt = mybir.dt
    f32, bf16, f8 = dt.float32, dt.bfloat16, dt.float8e4
    DR = mybir.MatmulPerfMode.DoubleRow

    nc = bacc.Bacc("TRN2", target_bir_lowering=False, debug=False,
                   num_devices=N_CORES)

    # DRAM inputs. W tiles for stationary use (qT/kT/skT) are oc-major:
    # [oc, p, dc, (lo,hi), col]; wv for moving use (v/side-v) is dc-major:
    # [p, dc, (lo,hi), cols]. x/g are transposed hi/lo: [p, dc, (hi,lo), tok].
    x8_d = nc.declare_dram_parameter("x8", [128, 8, 2, KV], f8, isOutput=False)
    g8_d = nc.declare_dram_parameter("g8", [128, 8, 2, G], f8, isOutput=False)
    wq_d = nc.declare_dram_parameter("wq8", [8, 128, 8, 2, 128], f8,
                                     isOutput=False)
    wk_d = nc.declare_dram_parameter("wk8", [8, 128, 8, 2, 128], f8,
                                     isOutput=False)
    wv_d = nc.declare_dram_parameter("wv8", [128, 8, 2, D], f8, isOutput=False)
    wo_d = nc.declare_dram_parameter("wo", [D, D], bf16, isOutput=False)
    msk_d = nc.declare_dram_parameter("maskT", [BL, NB, 2, BL], bf16,
                                      isOutput=False)
    out_d = nc.declare_dram_parameter("out", [TOK, D], bf16, isOutput=True)

    Exp = mybir.ActivationFunctionType.Exp

    with tile.TileContext(nc) as tc:
        with (
            tc.tile_pool(name="per", bufs=1) as per,
            tc.tile_pool(name="att", bufs=2) as att,
            tc.tile_pool(name="osb", bufs=2) as osb,
        ):
            # ---- persistent SBUF tiles ----
            x8 = per.tile([128, 8, 2, KV], f8, tag="x8")
            wq8 = per.tile([128, 8, 8, 2, 128], f8, tag="wq_yT")  # dies -> yTf
            wk8 = per.tile([128, 8, 8, 2, 128], f8, tag="wk8")
            wv8 = per.tile([128, 8, 2, D], f8, tag="wv8")
            g8 = per.tile([128, 8, 2, G], f8, tag="g8")
            wo_sb = per.tile([128, 8, D], bf16, tag="wo")
            msk_sb = per.tile([128, NB, 2, BL], bf16, tag="msk")
            qT = per.tile([128, 8, TOK], bf16, tag="qT")
            kT = per.tile([128, 8, KV], bf16, tag="kT")
            vaug = per.tile([128, 10, H, DH + 1], bf16, tag="vaug")
            skT = per.tile([128, 8, G], bf16, tag="skT")
            svaug = per.tile([128, 2, H, DH + 1], bf16, tag="svaug")

            # DMA issue order = consumption order so compute chases the
            # stream: wq8[0], x8 chunks, rest of wq8, wv8, g8, wk8, msk, wo.
            nc.sync.dma_start(out=wq8[:, 0], in_=wq_d.ap()[0])
            for dc in range(8):
                nc.sync.dma_start(out=x8[:, dc], in_=x8_d.ap()[:, dc])
            for oc in range(1, 8):
                nc.sync.dma_start(out=wq8[:, oc], in_=wq_d.ap()[oc])
            for dc in range(8):
                nc.sync.dma_start(out=wv8[:, dc], in_=wv_d.ap()[:, dc])
            nc.sync.dma_start(out=g8, in_=g8_d.ap())
            for oc in range(8):
                nc.sync.dma_start(out=wk8[:, oc], in_=wk_d.ap()[oc])
            nc.sync.dma_start(out=msk_sb, in_=msk_d.ap())
            nc.sync.dma_start(out=wo_sb,
                              in_=wo_d.ap().rearrange("(c p) d -> p c d", p=128))

            def split_mm(pp, w_pair, w_single, x_pair, x_single, n):
                """Emit the 12-instruction split-fp8 product into psum pp.

                w_pair(a): lhsT [128,2,M] hi chunks (2a, 2a+1)
                w_single(A): lhsT [128,2,M] (lo,hi) of chunk A
                x_pair(a): rhs [128,2,N] hi chunks (2a, 2a+1)
                x_single(A): rhs [128,2,N] (hi,lo) of chunk A
                """
                for a in range(4):
                    nc.tensor.matmul(pp, w_pair(a), x_pair(a),
                                     start=(a == 0), stop=False, perf_mode=DR)
                for A in range(8):
                    nc.tensor.matmul(pp, w_single(A), x_single(A),
                                     start=False, stop=(A == 7), perf_mode=DR)

            # ---- qT projection (q tokens = kv rows 128..1152) ----
            with tc.tile_pool(name="pspa", bufs=3, space="PSUM") as psp:
                for oc in range(8):
                    for tch in range(2):
                        ts_ = 128 + tch * 512
                        pp = psp.tile([128, 512], f32, tag="pj")
                        split_mm(
                            pp,
                            lambda a, oc=oc: wq8[:, oc, 2 * a:2 * a + 2, 1, :],
                            lambda A, oc=oc: wq8[:, oc, A, :, :],
                            lambda a, t=ts_: x8[:, 2 * a:2 * a + 2, 0, t:t + 512],
                            lambda A, t=ts_: x8[:, A, :, t:t + 512], 512)
                        nc.vector.tensor_scalar_mul(
                            qT[:, oc, tch * 512:(tch + 1) * 512], pp, QS)
                # ---- v natural (augmented with ones column) ----
                for t in range(10):
                    for j in range(2):
                        pp = psp.tile([128, 512], f32, tag="pj")
                        split_mm(
                            pp,
                            lambda a, t=t: x8[:, 2 * a:2 * a + 2, 0,
                                              t * 128:(t + 1) * 128],
                            lambda A, t=t: x8[:, A, :, t * 128:(t + 1) * 128],
                            lambda a, j=j: wv8[:, 2 * a:2 * a + 2, 1,
                                               j * 512:(j + 1) * 512],
                            lambda A, j=j: wv8[:, A, :, j * 512:(j + 1) * 512],
                            512)
                        nc.scalar.mul(
                            vaug[:, t, 8 * j:8 * (j + 1), 0:DH],
                            pp.rearrange("p (h d) -> p h d", h=8), KS)
                    nc.vector.memset(vaug[:, t, :, DH:DH + 1], 1.0)
                # ---- side kT ----
                for oc in range(8):
                    pp = psp.tile([128, 512], f32, tag="pj")
                    split_mm(
                        pp[:, :G],
                        lambda a, oc=oc: wk8[:, oc, 2 * a:2 * a + 2, 1, :],
                        lambda A, oc=oc: wk8[:, oc, A, :, :],
                        lambda a: g8[:, 2 * a:2 * a + 2, 0, :],
                        lambda A: g8[:, A, :, :], G)
                    nc.vector.tensor_scalar_mul(skT[:, oc, :], pp[:, :G], GS)
                # ---- side v (augmented) ----
                for gt_i in range(2):
                    for j in range(2):
                        pp = psp.tile([128, 512], f32, tag="pj")
                        split_mm(
                            pp,
                            lambda a, g=gt_i: g8[:, 2 * a:2 * a + 2, 0,
                                                 g * 128:(g + 1) * 128],
                            lambda A, g=gt_i: g8[:, A, :, g * 128:(g + 1) * 128],
                            lambda a, j=j: wv8[:, 2 * a:2 * a + 2, 1,
                                               j * 512:(j + 1) * 512],
                            lambda A, j=j: wv8[:, A, :, j * 512:(j + 1) * 512],
                            512)
                        nc.scalar.mul(
                            svaug[:, gt_i, 8 * j:8 * (j + 1), 0:DH],
                            pp.rearrange("p (h d) -> p h d", h=8), GS)
                    nc.vector.memset(svaug[:, gt_i, :, DH:DH + 1], 1.0)

            # ---- kT projection + attention, interleaved per head pair ----
            with tc.tile_pool(name="pkt", bufs=2, space="PSUM") as pkt, \
                 tc.tile_pool(name="plg", bufs=2, space="PSUM") as plg, \
                 tc.tile_pool(name="psg", bufs=1, space="PSUM") as psg, \
                 tc.tile_pool(name="pyt", bufs=2, space="PSUM") as pyt:
                yTf = per.tile([128, 8, TOK], bf16, tag="wq_yT")

                # local chunk order in lg/ul is (c0, c2, c1) so the two
                # masked chunks are a single contiguous [128, 2, 128] slice.
                coff = (0, 2, 1)

                def attn_head(h, stg):
                    oc, r0 = h // 2, 64 * (h % 2)
                    for nh in range(2):          # half = 4 blocks = 512 q
                        q4 = qT[r0:r0 + 64, oc, nh * 512:(nh + 1) * 512]
                        sg = psg.tile([128, 2, 512], f32, tag="sg",
                                      name=f"sg{h}_{nh}")
                        us = att.tile([128, 2, 512], bf16, tag="us", bufs=3,
                                      name=f"us{h}_{nh}")
                        for g in range(2):
                            nc.tensor.matmul(
                                sg[:, g, :],
                                skT[r0:r0 + 64, oc, g * 128:(g + 1) * 128],
                                q4, start=True, stop=True)
                        nc.scalar.activation(us, sg, Exp)
                        uls = []
                        for i in range(4):
                            n = nh * 4 + i
                            qs = qT[r0:r0 + 64, oc, n * 128:(n + 1) * 128]
                            lg = plg.tile([128, 3, 128], f32, tag="lg",
                                          name=f"lg{h}_{n}")
                            for c in range(3):
                                nc.tensor.matmul(
                                    lg[:, c, :],
                                    kT[r0:r0 + 64, oc,
                                       (n + coff[c]) * 128:(n + coff[c] + 1) * 128],
                                    qs, start=True, stop=True)
                            ul = att.tile([128, 3, 128], bf16, tag="ul", bufs=8,
                                          name=f"ul{h}_{n}")
                            nc.scalar.activation(ul, lg, Exp)
                            nc.vector.tensor_mul(ul[:, 0:2, :], ul[:, 0:2, :],
                                                 msk_sb[:, n, :, :])
                            uls.append(ul)
                        yt = pyt.tile([65, 512], f32, tag="yt",
                                      name=f"yt{h}_{nh}")
                        for i in range(4):
                            n = nh * 4 + i
                            for c in range(3):
                                nc.tensor.matmul(
                                    yt[:, i * 128:(i + 1) * 128],
                                    vaug[:, n + coff[c], h, :], uls[i][:, c, :],
                                    start=(i == 0 and c == 0), stop=False)
                        for g in range(2):
                            nc.tensor.matmul(
                                yt, svaug[:, g, h, :], us[:, g, :],
                                start=False, stop=(g == 1))
                        # normalize: recip of denom row, partition-broadcast,
                        # multiply rows 0..63
                        rc1 = att.tile([1, 512], bf16, tag="rc1", bufs=2,
                                       name=f"rc1{h}_{nh}")
                        with nc.allow_low_precision(reason="bf16 softmax recip"):
                            nc.vector.reciprocal(rc1, yt[64:65, :])
                        rcb = att.tile([64, 512], bf16, tag="rcb", bufs=2,
                                       name=f"rcb{h}_{nh}")
                        nc.gpsimd.partition_broadcast(rcb, rc1)
                        dst = (yTf[0:64, oc, nh * 512:(nh + 1) * 512]
                               if h % 2 == 0 else
                               stg[:, nh * 512:(nh + 1) * 512])
                        nc.vector.tensor_mul(dst, yt[0:64, :], rcb)

                for oc in range(8):
                    for ts_, te in ((0, 512), (512, 1024), (1024, 1280)):
                        pp = pkt.tile([128, 512], f32, tag="pj",
                                      name=f"ppk{oc}_{ts_}")
                        split_mm(
                            pp[:, :te - ts_],
                            lambda a, oc=oc: wk8[:, oc, 2 * a:2 * a + 2, 1, :],
                            lambda A, oc=oc: wk8[:, oc, A, :, :],
                            lambda a, ts_=ts_, te=te: x8[:, 2 * a:2 * a + 2, 0,
                                                         ts_:te],
                            lambda A, ts_=ts_, te=te: x8[:, A, :, ts_:te],
                            te - ts_)
                        nc.vector.tensor_scalar_mul(kT[:, oc, ts_:te],
                                                    pp[:, :te - ts_], KS)
                    attn_head(2 * oc, None)
                    stg = att.tile([64, TOK], bf16, tag="stg", bufs=2,
                                   name=f"stg{oc}")
                    attn_head(2 * oc + 1, stg)
                    nc.sync.dma_start(out=yTf[64:128, oc, :], in_=stg)

            # ---- output projection (bf16) ----
            with tc.tile_pool(name="pso", bufs=2, space="PSUM") as pso:
                for tt in range(8):
                    ot = osb.tile([128, D], bf16, tag="ot")
                    for j in range(2):
                        pp = pso.tile([128, 512], f32, tag="po")
                        for oc in range(8):
                            nc.tensor.matmul(
                                pp,
                                yTf[:, oc, tt * 128:(tt + 1) * 128],
                                wo_sb[:, oc, 512 * j:512 * (j + 1)],
                                start=(oc == 0), stop=(oc == 7))
                        nc.scalar.copy(ot[:, 512 * j:512 * (j + 1)], pp)
                    nc.sync.dma_start(out=out_d[tt * 128:(tt + 1) * 128, :],
                                      in_=ot)

    nc.compile()
    return nc


def _split8(a, scale):
    """Return (hi, lo) e4m3 split of a*scale (f32 in, f8 out)."""
    a = np.asarray(a, np.float32) * scale
    hi = a.astype(F8)
    lo = (a - hi.astype(np.float32)).astype(F8)
    return hi, lo


def _host_inputs(x, Wq, Wk, Wv, Wo):
    """Build the 8 per-core input maps (numpy, fp8/bf16 as the device expects)."""
    x = np.asarray(x, np.float32)
    # full per-example transposed x, hi/lo split: [B][D, L]
    xT = x.transpose(0, 2, 1)                      # [B, D, L]
    xh, xl = _split8(xT, SX)
    g = x.reshape(B, G, TPB, D).sum(2)             # [B, G, D] f32
    gT = g.transpose(0, 2, 1)                      # [B, D, G]
    gh, gl = _split8(gT, SG)

    def w_ocmajor(W, scale):
        # [D, D] -> [8oc, 128p, 8dc, 2(lo,hi), 128col] fp8
        h, lo = _split8(W, scale)
        out = np.empty((8, 128, 8, 2, 128), F8)
        W4 = np.stack([lo, h], axis=0)             # [2, D, D]
        for oc in range(8):
            for dc in range(8):
                blk = W4[:, dc * 128:(dc + 1) * 128, oc * 128:(oc + 1) * 128]
                out[oc, :, dc, :, :] = blk.transpose(1, 0, 2)
        return out

    def w_dcmajor(W, scale):
        # [D, D] -> [128p, 8dc, 2(lo,hi), D cols] fp8
        h, lo = _split8(W, scale)
        out = np.empty((128, 8, 2, D), F8)
        for dc in range(8):
            out[:, dc, 0, :] = lo[dc * 128:(dc + 1) * 128, :]
            out[:, dc, 1, :] = h[dc * 128:(dc + 1) * 128, :]
        return out

    wq8 = w_ocmajor(np.asarray(Wq, np.float32).reshape(D, D) / np.sqrt(DH),
                    SW * 8.0)
    wk8 = w_ocmajor(np.asarray(Wk, np.float32).reshape(D, D), SW)
    wv8 = w_dcmajor(np.asarray(Wv, np.float32).reshape(D, D), SW)
    wo = np.asarray(Wo, np.float32).reshape(D, D).astype(BF16)

    in_maps = []
    for c in range(N_CORES):
        b, s = c // 4, c % 4
        S0 = s * TOK
        blk0 = S0 // BL
        a0 = S0 - BL
        lo_t, hi_t = max(a0, 0), min(a0 + KV, L)
        x8 = np.zeros((128, 8, 2, KV), F8)
        for dc in range(8):
            x8[:, dc, 0, lo_t - a0:hi_t - a0] = \
                xh[b, dc * 128:(dc + 1) * 128, lo_t:hi_t]
            x8[:, dc, 1, lo_t - a0:hi_t - a0] = \
                xl[b, dc * 128:(dc + 1) * 128, lo_t:hi_t]
        g8 = np.empty((128, 8, 2, G), F8)
        for dc in range(8):
            g8[:, dc, 0, :] = gh[b, dc * 128:(dc + 1) * 128, :]
            g8[:, dc, 1, :] = gl[b, dc * 128:(dc + 1) * 128, :]
        # maskT[k, n, c01, q] for local chunks (c0, c2) only
        k_ = np.arange(BL)[:, None, None, None]
        n_ = np.arange(NB)[None, :, None, None]
        c_ = np.array([0, 2])[None, None, :, None]
        q_ = np.arange(BL)[None, None, None, :]
        rel = (c_ * BL + k_) - BL - q_
        kpos = (blk0 + n_ - 1) * BL + c_ * BL + k_
        valid = (np.abs(rel) <= BL - 1) & (kpos >= 0) & (kpos < L)
        in_maps.append({
            "x8": x8, "g8": g8,
            "wq8": wq8, "wk8": wk8, "wv8": wv8, "wo": wo,
            "maskT": valid.astype(BF16),
        })
    return in_maps


_RUNNER = None


def _make_runner(nc):
    """Build the PJRT executable once; returns fn(in_maps) -> per-core outs."""
    import jax
    import numpy as _np
    from jax.sharding import Mesh, PartitionSpec
    from jax.experimental.shard_map import shard_map
    import concourse.mybir as mybir
    from concourse import bass2jax

    bass2jax.install_neuronx_cc_hook()
    partition_name = (nc.partition_id_tensor.name
                      if nc.partition_id_tensor else None)
    in_names, out_names, out_avals = [], [], []
    for alloc in nc.m.functions[0].allocations:
        if not isinstance(alloc, mybir.MemoryLocationSet):
            continue
        name = alloc.memorylocations[0].name
        if alloc.kind == "ExternalInput":
            if name != partition_name:
                in_names.append(name)
        elif alloc.kind == "ExternalOutput":
            out_avals.append(jax.core.ShapedArray(
                tuple(alloc.tensor_shape), mybir.dt.np(alloc.dtype)))
            out_names.append(name)
    n_params = len(in_names)
    all_names = in_names + out_names
    if partition_name is not None:
        all_names.append(partition_name)
    donate = tuple(range(n_params, n_params + len(out_names)))

    def _body(*args):
        operands = list(args)
        if partition_name is not None:
            operands.append(bass2jax.partition_id_tensor())
        return tuple(bass2jax._bass_exec_p.bind(
            *operands, out_avals=tuple(out_avals), in_names=tuple(all_names),
            out_names=tuple(out_names), lowering_input_output_aliases=(),
            sim_require_finite=True, sim_require_nnan=True, nc=nc))

    devices = jax.devices()[:N_CORES]
    mesh = Mesh(_np.asarray(devices), ("core",))
    specs = (PartitionSpec("core"),) * (n_params + len(out_names))
    sharded = jax.jit(
        shard_map(_body, mesh=mesh, in_specs=specs,
                  out_specs=(PartitionSpec("core"),) * len(out_names),
                  check_rep=False),
        donate_argnums=donate, keep_unused=True)

    def run(in_maps):
        concat_in = [
            _np.concatenate([_np.asarray(in_maps[c][k]) for c in range(N_CORES)],
                            axis=0)
            for k in in_names
        ]
        concat_zeros = [_np.zeros((N_CORES * a.shape[0], *a.shape[1:]), a.dtype)
                        for a in out_avals]
        outs = sharded(*concat_in, *concat_zeros)
        return [
            {k: _np.asarray(outs[i]).reshape(N_CORES, *out_avals[i].shape)[c]
             for i, k in enumerate(out_names)}
            for c in range(N_CORES)
        ]

    return run


def kernel(x, Wq, Wk, Wv, Wo):
    global _PROG, _RUNNER
    if _RUNNER is None:
        _PROG = _build_program()
        _RUNNER = _make_runner(_PROG)
    in_maps = _host_inputs(np.asarray(x, np.float32), np.asarray(Wq, np.float32),
                           np.asarray(Wk, np.float32), np.asarray(Wv, np.float32),
                           np.asarray(Wo, np.float32))
    results = _RUNNER(in_maps)
    out = np.empty((B, L, D), np.float32)
    for c in range(N_CORES):
        b, s = c // 4, c % 4
        out[b, s * TOK:(s + 1) * TOK] = results[c]["out"].astype(np.float32)
    return out
